# revision 29
# baseline (speedup 1.0000x reference)
"""Trainium2 Bass kernel for AngularFeaturePropagation (retrieval_knn).

Computation per batch element b (one NeuronCore per b, B=8 = n_cores):
  idx[n]  = argmin_m sqrt((lt[n]-ht[m])^2 + (lp[n]-hp[m])^2)      n<8192, m<2048
  interp  = high_feats[:, idx]                                     [128, 8192]
  cat     = [low_feats; interp]                                    [192, 8192]
  y0 = W0 @ cat  -> BN(global batch stats, over all cores) -> ReLU
  y1 = W1 @ h0   -> BN -> ReLU -> out                              [128, 8192]

Device algorithm:
  - scores s[n,m] = 2*lt*ht + 2*lp*hp - (ht^2+hp^2)  (= -dist^2 + const(n))
    via PE matmul (K=3), PSUM [128 queries, 2048 cands] per query tile.
  - DVE max (top-8) + max_index per tile -> approx argmax. Per 8-tile
    batch, the top-2 indices are exported and their coord-record gathers
    (dma_gather, 256B padded rows, <=1024 descriptors per call to fit the
    SWDGE carveout) launch immediately so they overlap the remaining DVE
    scans.
  - exact rescore of top-2 (PE expansion has ~1e-6 rounding error vs the
    reference's direct (d_theta^2+d_phi^2)): compare (2lt-2ht)^2+(2lp-2hp)^2
    exactly in token-major layout, select; zero argmin flips vs reference.
  - feature gather via GPSIMD ap_gather (channel-major, no transpose),
    chunked 4x so MLP layer-0 chunks can start after the first slice.
  - MLP on PE; BN batch stats via bn_stats/bn_aggr + AllReduce of
    (mean_i, E[x^2]_i); biases dropped (BN shift-invariant); affine+ReLU
    fused into one ScalarE activation per layer.
"""

import sys

if '/opt/trn_rl_repo' not in sys.path:
    sys.path.insert(0, '/opt/trn_rl_repo')

import numpy as np

import concourse.bass as bass
import concourse.bacc as bacc
import concourse.tile as tile
import concourse.mybir as mybir
from concourse import bass_utils, library_config

F32 = mybir.dt.float32
U16 = mybir.dt.uint16
I16 = mybir.dt.int16
AF = mybir.ActivationFunctionType
OP = mybir.AluOpType

B, N, M, C1, C2 = 8, 8192, 2048, 64, 128
NT = N // 128          # 64 query tiles
NCH = N // 512         # 16 MLP chunks
EPS = 1e-5


def build(num_devices=8):
    nc = bacc.Bacc("TRN2", target_bir_lowering=False, debug=False, num_devices=num_devices)

    # ---- per-core DRAM I/O ----
    d_lt = nc.dram_tensor("lt", [N], F32, kind="ExternalInput")
    d_lp = nc.dram_tensor("lp", [N], F32, kind="ExternalInput")
    d_ht = nc.dram_tensor("ht", [M], F32, kind="ExternalInput")
    d_hp = nc.dram_tensor("hp", [M], F32, kind="ExternalInput")
    d_lf = nc.dram_tensor("lf", [C1, N], F32, kind="ExternalInput")
    d_hf = nc.dram_tensor("hf", [C2, M], F32, kind="ExternalInput")
    d_w0lot = nc.dram_tensor("w0lot", [C1, 128], F32, kind="ExternalInput")
    d_w0hit = nc.dram_tensor("w0hit", [C2, 128], F32, kind="ExternalInput")
    d_w1t = nc.dram_tensor("w1t", [128, 128], F32, kind="ExternalInput")
    d_g0 = nc.dram_tensor("g0", [128, 1], F32, kind="ExternalInput")
    d_be0 = nc.dram_tensor("be0", [128, 1], F32, kind="ExternalInput")
    d_g1 = nc.dram_tensor("g1", [128, 1], F32, kind="ExternalInput")
    d_be1 = nc.dram_tensor("be1", [128, 1], F32, kind="ExternalInput")
    d_ones = nc.dram_tensor("ones", [1, N], F32, kind="ExternalInput")
    d_out = nc.dram_tensor("out", [128, N], F32, kind="ExternalOutput")

    with tile.TileContext(nc) as tc:
        with (
            tc.tile_pool(name="persist", bufs=1) as persist,
            tc.tile_pool(name="dram", bufs=1, space="DRAM") as dram,
            tc.tile_pool(name="big", bufs=3) as bigp,
            tc.tile_pool(name="small", bufs=2) as small,
        ):
            # ---------------- stage 0: loads & prep ----------------
            hf_sb = persist.tile([C2, M], F32)
            nc.sync.dma_start(hf_sb[:], d_hf.ap())
            nc.gpsimd.load_library(library_config.ap_gather)
            w0lot = persist.tile([C1, 128], F32)
            nc.sync.dma_start(w0lot[:], d_w0lot.ap())
            w0hit = persist.tile([C2, 128], F32)
            nc.sync.dma_start(w0hit[:], d_w0hit.ap())
            w1t = persist.tile([128, 128], F32)
            nc.sync.dma_start(w1t[:], d_w1t.ap())
            g0 = persist.tile([128, 1], F32)
            nc.sync.dma_start(g0[:], d_g0.ap())
            be0 = persist.tile([128, 1], F32)
            nc.sync.dma_start(be0[:], d_be0.ap())
            g1 = persist.tile([128, 1], F32)
            nc.sync.dma_start(g1[:], d_g1.ap())
            be1 = persist.tile([128, 1], F32)
            nc.sync.dma_start(be1[:], d_be1.ap())

            # query features for PE: rows [lt, lp, ones]
            qfeat = persist.tile([3, N], F32)
            nc.sync.dma_start(qfeat[0:1, :], d_lt.ap().rearrange("(o n) -> o n", o=1))
            nc.sync.dma_start(qfeat[1:2, :], d_lp.ap().rearrange("(o n) -> o n", o=1))
            nc.sync.dma_start(qfeat[2:3, :], d_ones.ap())

            # token-major query coords [128, 64]: token n = t*128 + p
            lt_tok = persist.tile([128, NT], F32)
            nc.sync.dma_start(lt_tok[:], d_lt.ap().rearrange("(t p) -> p t", p=128))
            lp_tok = persist.tile([128, NT], F32)
            nc.sync.dma_start(lp_tok[:], d_lp.ap().rearrange("(t p) -> p t", p=128))

            # candidate features [3, 2048]: rows [2ht, 2hp, -(ht^2+hp^2)]
            # computed in [128, 16] layout, bounced through DRAM.
            hco = small.tile([128, M // 128], F32)
            nc.sync.dma_start(hco[:], d_ht.ap().rearrange("(p f) -> p f", p=128))
            hpo = small.tile([128, M // 128], F32)
            nc.sync.dma_start(hpo[:], d_hp.ap().rearrange("(p f) -> p f", p=128))
            c0 = small.tile([128, M // 128], F32)
            nc.vector.tensor_scalar_mul(c0[:], hco[:], 2.0)
            c1 = small.tile([128, M // 128], F32)
            nc.vector.tensor_scalar_mul(c1[:], hpo[:], 2.0)
            sq = small.tile([128, M // 128], F32)
            nc.vector.tensor_mul(sq[:], hco[:], hco[:])
            sq2 = small.tile([128, M // 128], F32)
            nc.vector.tensor_mul(sq2[:], hpo[:], hpo[:])
            ssum = small.tile([128, M // 128], F32)
            nc.vector.tensor_add(ssum[:], sq[:], sq2[:])
            c2t = small.tile([128, M // 128], F32)
            nc.vector.tensor_scalar_mul(c2t[:], ssum[:], -1.0)

            candfeat = persist.tile([3, M], F32)
            nc.sync.dma_start(candfeat[0:1, :], c0[:])
            nc.sync.dma_start(candfeat[1:2, :], c1[:])
            nc.sync.dma_start(candfeat[2:3, :], c2t[:])

            # rescore record table [2048, 64]: row m = [2ht[m], 2hp[m], pad...]
            d_tbl = dram.tile([M, 64], F32)
            ztile = small.tile([128, M * 64 // 128], F32)
            nc.vector.memset(ztile[:], 0.0)
            nc.sync.dma_start(d_tbl[:].rearrange("(p a) f -> p (a f)", p=128), ztile[:])
            c01 = small.tile([128, 2 * (M // 128)], F32)
            c01v = c01[:].rearrange("p (f o) -> p f o", o=2)
            nc.vector.tensor_copy(c01v[:, :, 0:1], c0[:].rearrange("p (f o) -> p f o", o=1))
            nc.vector.tensor_copy(c01v[:, :, 1:2], c1[:].rearrange("p (f o) -> p f o", o=1))
            nc.sync.dma_start(d_tbl[:, 0:2].rearrange("(p f) o -> p f o", p=128), c01v)

            # ---------------- stage 1: scores + approx argmax ----------------
            idx8 = persist.tile([128, NT * 8], U16)  # top-8 indices per tile
            d_i1 = dram.tile([N], U16)
            d_i2 = dram.tile([N], U16)
            i1w = small.tile([128, N // 16], U16)
            i2w = small.tile([128, N // 16], U16)
            CK = 1024  # swdge descriptor carveout is 1024 descs
            rec1 = bigp.tile([128, NT, 64], F32, tag="big")
            rec2 = bigp.tile([128, NT, 64], F32, tag="big")
            idx8v = idx8[:].rearrange("p (t k) -> p t k", k=8)
            with (
                tc.tile_pool(name="spsum", bufs=2, space="PSUM") as spsum,
                tc.tile_pool(name="sc8", bufs=4) as sc8,
            ):
                for t in range(NT):
                    ps = spsum.tile([128, M], F32)
                    for k in range(4):
                        nc.tensor.matmul(
                            ps[:, 512 * k:512 * (k + 1)],
                            qfeat[:, 128 * t:128 * (t + 1)],
                            candfeat[:, 512 * k:512 * (k + 1)],
                            start=True, stop=True,
                        )
                    v8 = sc8.tile([128, 8], F32)
                    nc.vector.max(v8[:], ps[:])
                    nc.vector.max_index(idx8[:, 8 * t:8 * t + 8], v8[:], ps[:])
                    if t % 8 == 7:
                        # batch j of 8 tiles complete: export its indices and
                        # launch its gather chunk so the gathers overlap the
                        # remaining stage-1 DVE scans.
                        j = t // 8
                        nc.sync.dma_start(
                            d_i1[1024 * j:1024 * (j + 1)].rearrange("(t p o) -> p t o", p=128, o=1),
                            idx8v[:, 8 * j:8 * (j + 1), 0:1])
                        nc.sync.dma_start(
                            d_i2[1024 * j:1024 * (j + 1)].rearrange("(t p o) -> p t o", p=128, o=1),
                            idx8v[:, 8 * j:8 * (j + 1), 1:2])
                        for g in range(8):
                            nc.sync.dma_start(
                                i1w[16 * g:16 * (g + 1), 64 * j:64 * (j + 1)],
                                d_i1[1024 * j:1024 * (j + 1)].rearrange("(s p) -> p s", p=16))
                            nc.sync.dma_start(
                                i2w[16 * g:16 * (g + 1), 64 * j:64 * (j + 1)],
                                d_i2[1024 * j:1024 * (j + 1)].rearrange("(s p) -> p s", p=16))
                        nc.gpsimd.dma_gather(
                            rec1[:, 8 * j:8 * (j + 1), :], d_tbl[:],
                            i1w[:, 64 * j:64 * (j + 1)].bitcast(I16),
                            num_idxs=CK, num_idxs_reg=CK, elem_size=64,
                        )
                        nc.gpsimd.dma_gather(
                            rec2[:, 8 * j:8 * (j + 1), :], d_tbl[:],
                            i2w[:, 64 * j:64 * (j + 1)].bitcast(I16),
                            num_idxs=CK, num_idxs_reg=CK, elem_size=64,
                        )

            # ---------------- stage 2: exact top-2 rescore ----------------

            # exact 4*dist^2 for both candidates, token-major [128, 64]
            lt2 = small.tile([128, NT], F32)
            nc.vector.tensor_scalar_mul(lt2[:], lt_tok[:], 2.0)
            lp2 = small.tile([128, NT], F32)
            nc.vector.tensor_scalar_mul(lp2[:], lp_tok[:], 2.0)

            def exact_d2(rec, tag):
                da = small.tile([128, NT], F32, tag=tag + "a")
                nc.vector.tensor_sub(da[:], lt2[:], rec[:, :, 0])
                dasq = small.tile([128, NT], F32, tag=tag + "b")
                nc.vector.tensor_mul(dasq[:], da[:], da[:])
                db = small.tile([128, NT], F32, tag=tag + "c")
                nc.vector.tensor_sub(db[:], lp2[:], rec[:, :, 1])
                dbsq = small.tile([128, NT], F32, tag=tag + "d")
                nc.vector.tensor_mul(dbsq[:], db[:], db[:])
                e = small.tile([128, NT], F32, tag=tag + "e")
                nc.vector.tensor_add(e[:], dasq[:], dbsq[:])
                return e

            e1 = exact_d2(rec1, "r1")
            e2 = exact_d2(rec2, "r2")

            i1t = small.tile([128, NT], U16)
            nc.sync.dma_start(i1t[:], d_i1[:].rearrange("(t p) -> p t", p=128))
            i2t = small.tile([128, NT], U16)
            nc.sync.dma_start(i2t[:], d_i2[:].rearrange("(t p) -> p t", p=128))
            i1f = small.tile([128, NT], F32)
            nc.vector.tensor_copy(i1f[:], i1t[:])
            i2f = small.tile([128, NT], F32)
            nc.vector.tensor_copy(i2f[:], i2t[:])

            m_lt = small.tile([128, NT], mybir.dt.uint8)
            nc.vector.tensor_tensor(m_lt[:], e2[:], e1[:], op=OP.is_lt)
            m_eq = small.tile([128, NT], mybir.dt.uint8)
            nc.vector.tensor_tensor(m_eq[:], e2[:], e1[:], op=OP.is_equal)
            imin = small.tile([128, NT], F32)
            nc.vector.tensor_tensor(imin[:], i1f[:], i2f[:], op=OP.min)
            pick0 = small.tile([128, NT], F32)
            nc.vector.select(pick0[:], m_lt[:], i2f[:], i1f[:])
            fidx = small.tile([128, NT], F32)
            nc.vector.select(fidx[:], m_eq[:], imin[:], pick0[:])

            fidx_u = small.tile([128, NT], U16)
            nc.vector.tensor_copy(fidx_u[:], fidx[:])
            d_fi = dram.tile([N], U16)
            nc.sync.dma_start(d_fi[:].rearrange("(t p) -> p t", p=128), fidx_u[:])
            fiw = small.tile([128, N // 16], U16)
            for g in range(8):
                nc.sync.dma_start(fiw[16 * g:16 * (g + 1), :], d_fi[:].rearrange("(s p) -> p s", p=16))

            # ---------------- stage 3: feature gather ----------------
            interp = bigp.tile([C2, N], F32, tag="big")
            interp3 = interp[:].rearrange("p (m d) -> p m d", d=1)
            hf3 = hf_sb[:].rearrange("p (m d) -> p m d", d=1)
            for q in range(4):
                nc.gpsimd.ap_gather(
                    interp3[:, 2048 * q:2048 * (q + 1), :], hf3,
                    fiw[:, 128 * q:128 * (q + 1)].bitcast(I16),
                    channels=128, num_elems=M, d=1, num_idxs=2048,
                )

            # ---------------- stage 4/5: MLP + BN + ReLU ----------------
            d_ccin = dram.tile([128, 2], F32)
            d_ccout = dram.tile([128, 2], F32)
            d_ccin1 = dram.tile([128, 2], F32)
            d_ccout1 = dram.tile([128, 2], F32)

            def bn_apply(y_sb, st, gam, bet, d_in, d_out, out_sb, relu=True, store_to=None):
                ag = small.tile([128, 2], F32, tag="ag")
                nc.vector.bn_aggr(ag[:], st[:])
                msq = small.tile([128, 1], F32, tag="msq")
                nc.vector.tensor_mul(msq[:], ag[:, 0:1], ag[:, 0:1])
                cc = small.tile([128, 2], F32, tag="cc")
                nc.vector.tensor_copy(cc[:, 0:1], ag[:, 0:1])
                nc.vector.tensor_add(cc[:, 1:2], ag[:, 1:2], msq[:])
                nc.sync.dma_start(d_in[:], cc[:])
                if num_devices > 1:
                    nc.gpsimd.collective_compute(
                        "AllReduce", OP.add,
                        replica_groups=[list(range(num_devices))],
                        ins=[d_in[:].opt()], outs=[d_out[:].opt()],
                    )
                else:
                    nc.sync.dma_start(d_out[:], d_in[:])
                ccr = small.tile([128, 2], F32, tag="ccr")
                nc.sync.dma_start(ccr[:], d_out[:])
                mu = small.tile([128, 1], F32, tag="mu")
                nc.vector.tensor_scalar_mul(mu[:], ccr[:, 0:1], 1.0 / num_devices)
                e2g = small.tile([128, 1], F32, tag="e2g")
                nc.vector.tensor_scalar_mul(e2g[:], ccr[:, 1:2], 1.0 / num_devices)
                musq = small.tile([128, 1], F32, tag="musq")
                nc.vector.tensor_mul(musq[:], mu[:], mu[:])
                var = small.tile([128, 1], F32, tag="var")
                nc.vector.tensor_sub(var[:], e2g[:], musq[:])
                vpe = small.tile([128, 1], F32, tag="vpe")
                nc.vector.tensor_scalar_add(vpe[:], var[:], EPS)
                sd = small.tile([128, 1], F32, tag="sd")
                nc.scalar.activation(sd[:], vpe[:], AF.Sqrt)
                rs = small.tile([128, 1], F32, tag="rs")
                nc.vector.reciprocal(rs[:], sd[:])
                sc = small.tile([128, 1], F32, tag="sc")
                nc.vector.tensor_mul(sc[:], gam[:], rs[:])
                msc = small.tile([128, 1], F32, tag="msc")
                nc.vector.tensor_mul(msc[:], mu[:], sc[:])
                sh = small.tile([128, 1], F32, tag="sh")
                nc.vector.tensor_sub(sh[:], bet[:], msc[:])
                if store_to is None:
                    nc.scalar.activation(
                        out_sb[:], y_sb[:], AF.Relu if relu else AF.Copy,
                        bias=sh[:], scale=sc[:],
                    )
                else:
                    for q in range(4):
                        s_ = slice(2048 * q, 2048 * (q + 1))
                        nc.scalar.activation(
                            out_sb[:, s_], y_sb[:, s_], AF.Relu if relu else AF.Copy,
                            bias=sh[:], scale=sc[:],
                        )
                        nc.sync.dma_start(store_to[:, s_], out_sb[:, s_])

            with tc.tile_pool(name="mpsum", bufs=4, space="PSUM") as mpsum:
                # layer 0
                y0 = bigp.tile([128, N], F32, tag="big")
                st0 = persist.tile([128, NCH * 6], F32)
                lfq = []
                for q in range(4):
                    lfqt = small.tile([C1, 2048], F32, tag="lfq")
                    lfq.append(lfqt)
                    nc.sync.dma_start(lfqt[:], d_lf.ap()[:, 2048 * q:2048 * (q + 1)])
                for c in range(NCH):
                    lfch = lfq[c // 4][:, 512 * (c % 4):512 * (c % 4 + 1)]
                    ps = mpsum.tile([128, 512], F32)
                    nc.tensor.matmul(ps[:], w0lot[:], lfch,
                                     start=True, stop=False)
                    nc.tensor.matmul(ps[:], w0hit[:], interp[:, 512 * c:512 * (c + 1)],
                                     start=False, stop=True)
                    nc.vector.bn_stats(st0[:, 6 * c:6 * (c + 1)], ps[:])
                    nc.scalar.activation(y0[:, 512 * c:512 * (c + 1)], ps[:], AF.Copy)

                h0 = bigp.tile([128, N], F32, tag="big")
                bn_apply(y0, st0, g0, be0, d_ccin, d_ccout, h0)

                # layer 1
                y1 = bigp.tile([128, N], F32, tag="big")
                st1 = persist.tile([128, NCH * 6], F32)
                for c in range(NCH):
                    ps = mpsum.tile([128, 512], F32)
                    nc.tensor.matmul(ps[:], w1t[:], h0[:, 512 * c:512 * (c + 1)],
                                     start=True, stop=True)
                    nc.vector.bn_stats(st1[:, 6 * c:6 * (c + 1)], ps[:])
                    nc.scalar.activation(y1[:, 512 * c:512 * (c + 1)], ps[:], AF.Copy)

                o_sb = bigp.tile([128, N], F32, tag="big")
                bn_apply(y1, st1, g1, be1, d_ccin1, d_ccout1, o_sb,
                         store_to=d_out.ap())

    nc.compile()
    return nc


_NC_CACHE = None


def _get_nc():
    global _NC_CACHE
    if _NC_CACHE is None:
        _NC_CACHE = build()
    return _NC_CACHE


def make_in_maps(inputs):
    lt = np.ascontiguousarray(inputs['low_theta'], np.float32)
    lp = np.ascontiguousarray(inputs['low_phi'], np.float32)
    lf = np.ascontiguousarray(inputs['low_feats'], np.float32)
    ht = np.ascontiguousarray(inputs['high_theta'], np.float32)
    hp = np.ascontiguousarray(inputs['high_phi'], np.float32)
    hf = np.ascontiguousarray(inputs['high_feats'], np.float32)
    W0 = np.asarray(inputs['W0'], np.float32)
    W1 = np.asarray(inputs['W1'], np.float32)
    w0lot = np.ascontiguousarray(W0[:, :C1].T)       # [64, 128]
    w0hit = np.ascontiguousarray(W0[:, C1:].T)       # [128, 128]
    w1t = np.ascontiguousarray(W1.T)                 # [128, 128]
    g0 = np.ascontiguousarray(np.asarray(inputs['g0'], np.float32).reshape(128, 1))
    be0 = np.ascontiguousarray(np.asarray(inputs['beta0'], np.float32).reshape(128, 1))
    g1 = np.ascontiguousarray(np.asarray(inputs['g1'], np.float32).reshape(128, 1))
    be1 = np.ascontiguousarray(np.asarray(inputs['beta1'], np.float32).reshape(128, 1))
    ones = np.ones((1, N), np.float32)

    in_maps = []
    for b in range(B):
        in_maps.append({
            "lt": lt[b], "lp": lp[b], "ht": ht[b], "hp": hp[b],
            "lf": lf[b], "hf": hf[b],
            "w0lot": w0lot, "w0hit": w0hit, "w1t": w1t,
            "g0": g0, "be0": be0, "g1": g1, "be1": be1,
            "ones": ones,
        })
    return in_maps


def kernel(**inputs):
    nc = _get_nc()
    in_maps = make_in_maps(inputs)
    res = bass_utils.run_bass_kernel_spmd(nc, in_maps, core_ids=list(range(B)))
    out = np.stack([res.results[b]["out"] for b in range(B)], axis=0)
    return out.astype(np.float32)
